# revision 48
# baseline (speedup 1.0000x reference)
# Longformer/BART encoder layer on 8 Trainium2 NeuronCores.
#
# Sharding: data-parallel over batch (2) x sequence-parallel (4 shards of
# 1024 tokens). Each core gets its shard's hidden states with a 256-token
# halo on each side (zero-padded at sequence edges), computes the full
# encoder layer for its 1024 tokens, and the host concatenates results.
#
# Precision/layout: QKV+O projections run fp8e4m3 DoubleRow (weights
# pre-scaled x16 host-side, descaled at PSUM eviction; contraction pairs
# packed [128, 2, n]); the FFN runs bf16 (fp8 there exceeds the error
# budget); attention scores run fp32r (self-loading matmuls -- the
# separate-ldweights path crashes at runtime when adjacent matmuls sit in
# different PE row groups) with bf16 probs/V. K and V stay resident in
# SBUF (no DRAM roundtrip). The additive key mask is folded into V
# multiplicatively (host ships exp(km); the fused ones-column denominator
# picks it up too), so each [keys, 512] score tile takes exactly one
# bias-free Exp and at most one two-chunk band select. FFN weights stream
# in packed contiguous layout; LayerNorm stats/broadcast/apply are
# interleaved into the FFN matmul stream so the in-order PE queue never
# stalls on LN; LN reductions ride the PE via ones-column matmuls.

from contextlib import ExitStack

import numpy as np

B, S, D, H, HD, FFN = 2, 4096, 1024, 16, 64, 4096
W = 256            # one-sided attention window
T = 1024           # tokens per core
TH = T + 2 * W     # halo'd tokens (1536)
NEG = -1e9
NCORES = 8
NB = T // 256      # query blocks per core (4)
NCH = TH // 128    # halo key chunks (12)

_CACHE = {}


def _build():
    import concourse.mybir as mybir
    import concourse.tile as tile
    from concourse import bacc

    F32, F32R = mybir.dt.float32, mybir.dt.float32r
    BF16 = mybir.dt.bfloat16
    AF = mybir.ActivationFunctionType
    ALU = mybir.AluOpType

    nc = bacc.Bacc("TRN2", target_bir_lowering=False, debug=False,
                   num_devices=NCORES)

    FP8 = mybir.dt.float8e4
    DR = mybir.MatmulPerfMode.DoubleRow
    # x8/w8: DoubleRow-packed fp8, [p, g*2F + i*F + n] = M[g*256+i*128+p, n]
    x8_d = nc.dram_tensor("x8", [128, 8 * TH], FP8, kind="ExternalInput")
    xc_d = nc.dram_tensor("xc", [D, T], BF16, kind="ExternalInput")
    wq_d = nc.dram_tensor("wq", [128, 8 * D], FP8, kind="ExternalInput")
    wk_d = nc.dram_tensor("wk", [128, 8 * D], FP8, kind="ExternalInput")
    wv_d = nc.dram_tensor("wv", [128, 8 * D], FP8, kind="ExternalInput")
    wo_d = nc.dram_tensor("wo", [128, 8 * D], FP8, kind="ExternalInput")
    w1_d = nc.dram_tensor("w1", [128, 32 * 1024], BF16, kind="ExternalInput")
    w2_d = nc.dram_tensor("w2", [128, 32 * 1024], BF16, kind="ExternalInput")
    # all [128, n] fp32 constants in one block: vmask(12) bvB(1040) then
    # bq bk bo b1 b2 g1 e1 g2 e2 (8+8+8+32+8+8+8+8+8 = 96)
    NCST = NCH + H * 65 + 96
    cst_d = nc.dram_tensor("cstb", [128, NCST], F32, kind="ExternalInput")
    onesP_d = nc.dram_tensor("onesP", [128, 1], F32R, kind="ExternalInput")
    onesF_d = nc.dram_tensor("onesF", [1, 128], F32R, kind="ExternalInput")
    yT_d = nc.dram_tensor("yT", [D, T], F32, kind="ExternalOutput")

    def ln_stats(psp, lnp, x_at, ones_x, ones_col, eps1, sfx=""):
        # x_at(m): [128, 512] AP (chunk m of this t-slice), 8 chunks.
        # ones_x matches x_at's dtype (bf16 or f32r); ones_col is f32r.
        # Returns (mrow, rrow): per-token mean and 1/std rows [1, 512].
        s1p = psp.tile([1, 512], F32, tag=f"lns1{sfx}", bufs=1,
                       name="lns1")
        s2p = psp.tile([1, 512], F32, tag=f"lns2{sfx}", bufs=1,
                       name="lns2")
        for m in range(8):
            nc.tensor.matmul(s1p[:], ones_x[:], x_at(m),
                             start=(m == 0), stop=(m == 7))
        for m in range(8):
            sq = lnp.tile([128, 512], F32R, tag="lnsq", bufs=4, name="lnsq")
            xm = x_at(m)
            if xm.dtype == F32R:
                xm = xm.bitcast(F32)
            if m % 2 == 0:
                nc.scalar.activation(sq[:], xm, AF.Square)
            else:
                nc.vector.tensor_mul(sq[:], xm, xm)
            nc.tensor.matmul(s2p[:], ones_col[:], sq[:],
                             start=(m == 0), stop=(m == 7))
        mrow = lnp.tile([1, 512], F32R, tag=f"lnmr{sfx}", bufs=1, name="lnmr")
        nc.scalar.activation(mrow[:], s1p[:], AF.Copy, scale=1.0 / D)
        a2 = lnp.tile([1, 512], F32, tag=f"lna2{sfx}", bufs=1, name="lna2")
        nc.scalar.activation(a2[:], s2p[:], AF.Copy, scale=1.0 / D)
        msq = lnp.tile([1, 512], F32, tag=f"lnms{sfx}", bufs=1, name="lnms")
        mf = mrow[:].bitcast(F32)
        nc.vector.tensor_mul(msq[:], mf, mf)
        vrow = lnp.tile([1, 512], F32, tag=f"lnvr{sfx}", bufs=1, name="lnvr")
        nc.vector.tensor_sub(vrow[:], a2[:], msq[:])
        srow = lnp.tile([1, 512], F32, tag=f"lnsr{sfx}", bufs=1, name="lnsr")
        nc.scalar.activation(srow[:], vrow[:], AF.Sqrt, bias=eps1[0:1, :])
        rrow = lnp.tile([1, 512], F32R, tag=f"lnrr{sfx}", bufs=1, name="lnrr")
        with nc.allow_low_precision(reason="fp32r rounding is fine here"):
            nc.vector.reciprocal(rrow[:], srow[:])
        return mrow, rrow

    def ln_bcast(psp, stats, ones_row, sfx=""):
        mrow, rrow = stats
        mb = psp.tile([128, 512], F32, tag=f"lnmb{sfx}", bufs=1, name="lnmb")
        nc.tensor.matmul(mb[:], ones_row[:], mrow[:])
        rb = psp.tile([128, 512], F32, tag=f"lnrb{sfx}", bufs=1, name="lnrb")
        nc.tensor.matmul(rb[:], ones_row[:], rrow[:])
        return mb, rb

    def ln_apply(lnp, x_at, g_t, e_t, out_at, mb, rb, ms=range(8)):
        for m in ms:
            dd = lnp.tile([128, 512], F32, tag="lnd", bufs=2, name="lnd")
            xm = x_at(m)
            if xm.dtype == F32R:
                xm = xm.bitcast(F32)
            nc.vector.tensor_sub(dd[:], xm, mb[:])
            tt = lnp.tile([128, 512], F32, tag="lnt", bufs=2, name="lnt")
            nc.vector.scalar_tensor_tensor(tt[:], dd[:], g_t[:, m:m + 1],
                                           rb[:], ALU.mult, ALU.mult)
            nc.vector.tensor_scalar_add(out_at(m), tt[:], e_t[:, m:m + 1])

    with tile.TileContext(nc) as tc, ExitStack() as ctx:
        cst = ctx.enter_context(tc.tile_pool(name="cst", bufs=1))
        qat = ctx.enter_context(tc.tile_pool(name="qat", bufs=1))

        cst_t = cst.tile([128, NCST], F32, name="cst_t")
        vmask = cst_t[:, 0:NCH]
        bvB = cst_t[:, NCH:NCH + H * 65]
        bt = {}
        off = NCH + H * 65
        for nm, ncol in [("bq", 8), ("bk", 8), ("bo", 8), ("b1", 32),
                         ("b2", 8), ("g1", 8), ("e1", 8), ("g2", 8),
                         ("e2", 8)]:
            bt[nm] = cst_t[:, off:off + ncol]
            off += ncol
        ones_col = cst.tile([128, 1], F32R, name="ones_col")
        nc.sync.dma_start(ones_col[:], onesP_d.ap())
        ones_colb = cst.tile([128, 1], BF16, name="ones_colb")
        nc.vector.tensor_copy(ones_colb[:], ones_col[:].bitcast(F32))
        ones_row = cst.tile([1, 128], F32R, name="ones_row")
        nc.sync.dma_start(ones_row[:], onesF_d.ap())
        eps1 = cst.tile([128, 1], F32, name="eps1")
        nc.vector.memset(eps1[:], 1e-5)

        qT = [qat.tile([128, T], F32R, tag=f"qT{m}", name=f"qT{m}")
              for m in range(8)]

        with tc.tile_pool(name="xtp", bufs=1) as xtp:
            xC = [xtp.tile([128, T], BF16, tag=f"xC{k}", name=f"xC{k}")
                  for k in range(8)]

            with tc.tile_pool(name="wp", bufs=12) as wp, \
                 tc.tile_pool(name="kvp", bufs=1) as kvp:
                kT = [kvp.tile([128, TH], F32R, tag=f"kT{m}",
                               name=f"kT{m}") for m in range(8)]
                vT = [kvp.tile([128, H * 65], BF16, tag=f"vT{t}",
                               name=f"vT{t}") for t in range(NCH)]

                # ---- QKV projections ----
                with tc.tile_pool(name="psp", bufs=4, space="PSUM") as psp, \
                     tc.tile_pool(name="x8p", bufs=1) as x8p:
                    x8 = []
                    for g in range(4):
                        t = x8p.tile([128, 2, TH], FP8, tag=f"x8{g}",
                                     name=f"x8{g}")
                        nc.sync.dma_start(t[:], x8_d.ap()[:, 2 * TH * g:
                                                          2 * TH * (g + 1)])
                        x8.append(t)
                    nc.sync.dma_start(cst_t[:], cst_d.ap())
                    wk_sb = []
                    for g in range(4):
                        t = wp.tile([128, 2, D], FP8, tag="w", name=f"wk{g}")
                        nc.sync.dma_start(t[:],
                                          wk_d.ap()[:, 2 * D * g:
                                                    2 * D * (g + 1)])
                        wk_sb.append(t)
                    for m in range(8):
                        for ts3 in range(3):
                            ps = psp.tile([128, 512], F32, tag="pj",
                                          name="pjk")
                            for g in range(4):
                                nc.tensor.matmul(
                                    ps[:],
                                    wk_sb[g][:, :, 128 * m:128 * (m + 1)],
                                    x8[g][:, :, 512 * ts3:512 * (ts3 + 1)],
                                    start=(g == 0), stop=(g == 3),
                                    perf_mode=DR)
                            nc.scalar.activation(
                                kT[m][:, 512 * ts3:512 * (ts3 + 1)],
                                ps[:], AF.Identity, scale=1.0 / 16,
                                bias=bt["bk"][:, m:m + 1])

                    wv_sb = []
                    for g in range(4):
                        t = wp.tile([128, 2, D], FP8, tag="w", name=f"wv{g}")
                        nc.sync.dma_start(t[:],
                                          wv_d.ap()[:, 2 * D * g:
                                                    2 * D * (g + 1)])
                        wv_sb.append(t)
                    for tm in range(NCH):
                        vt = vT[tm]
                        ones_dst = vt[:].rearrange(
                            "p (h c) -> p h c", c=65)[:, :, 64:65]
                        ones_src = bvB[:].rearrange(
                            "p (h c) -> p h c", c=65)[:, :, 64:65]
                        nc.vector.tensor_copy(ones_dst, ones_src)
                        for d2 in range(2):
                            ps = psp.tile([128, 512], F32, tag="pj",
                                          name="pjv")
                            for g in range(4):
                                nc.tensor.matmul(
                                    ps[:],
                                    x8[g][:, :, 128 * tm:128 * (tm + 1)],
                                    wv_sb[g][:, :, 512 * d2:512 * (d2 + 1)],
                                    start=(g == 0), stop=(g == 3),
                                    perf_mode=DR)
                            dst = vt[:, 520 * d2:520 * (d2 + 1)].rearrange(
                                "p (h c) -> p h c", c=65)[:, :, 0:64]
                            bsl = bvB[:, 520 * d2:520 * (d2 + 1)].rearrange(
                                "p (h c) -> p h c", c=65)[:, :, 0:64]
                            src = ps[:].rearrange("p (h c) -> p h c", c=64)
                            nc.vector.scalar_tensor_tensor(
                                dst, src, 1.0 / 16, bsl,
                                ALU.mult, ALU.add)
                        nc.vector.tensor_scalar_mul(vt[:], vt[:],
                                                    vmask[:, tm:tm + 1])

                    wq_sb = []
                    for g in range(4):
                        t = wp.tile([128, 2, D], FP8, tag="w", name=f"wq{g}")
                        nc.sync.dma_start(t[:],
                                          wq_d.ap()[:, 2 * D * g:
                                                    2 * D * (g + 1)])
                        wq_sb.append(t)
                    for m in range(8):
                        for t2 in range(2):
                            ps = psp.tile([128, 512], F32, tag="pj",
                                          name="pjq")
                            for g in range(4):
                                nc.tensor.matmul(
                                    ps[:],
                                    wq_sb[g][:, :, 128 * m:128 * (m + 1)],
                                    x8[g][:, :, W + 512 * t2:
                                          W + 512 * (t2 + 1)],
                                    start=(g == 0), stop=(g == 3),
                                    perf_mode=DR)
                            nc.scalar.activation(
                                qT[m][:, 512 * t2:512 * (t2 + 1)],
                                ps[:], AF.Identity, scale=0.125 / 16,
                                bias=bt["bq"][:, m:m + 1])

                # prefetch wo during attention
                wo_sb = []
                for g in range(4):
                    t = wp.tile([128, 2, D], FP8, tag="w", name=f"wo{g}")
                    nc.sync.dma_start(t[:],
                                      wo_d.ap()[:, 2 * D * g:2 * D * (g + 1)])
                    wo_sb.append(t)
                attn8 = []
                for g in range(4):
                    t = wp.tile([128, 2, T], FP8, tag=f"a8{g}", bufs=1,
                                name=f"a8{g}")
                    attn8.append(t)
                for k in range(8):
                    nc.sync.dma_start(xC[k][:],
                                      xc_d.ap()[128 * k:128 * (k + 1), :])

                # ---- sliding-window attention ----
                # Per-head score tiles: each [keys 128, 512] PSUM tile gets
                # both of its matmuls from ONE PE row group (mixed row
                # groups in a PSUM group crash walrus codegen at runtime).
                # The additive key mask is folded into V multiplicatively
                # (host ships exp(km); numerator and the fused ones-column
                # denominator both pick it up), so Exp needs no bias and
                # each tile takes one Exp and at most one band select.
                with tc.tile_pool(name="ptp", bufs=2) as ptp, \
                     tc.tile_pool(name="scp", bufs=4, space="PSUM") as scp, \
                     tc.tile_pool(name="pvp", bufs=2, space="PSUM") as pvp:
                    for h in range(H):
                        p0 = 64 * (h % 2)
                        hp = h // 2
                        for b in range(NB):
                            pts = []
                            for c2 in range(3):
                                sc = scp.tile([128, 512], F32, tag="sc",
                                              name="sc")
                                for half in range(2):
                                    c = 2 * c2 + half
                                    kc = 128 * (2 * b + c)
                                    nc.tensor.matmul(
                                        sc[:, 256 * half:256 * (half + 1)],
                                        kT[hp][p0:p0 + 64, kc:kc + 128],
                                        qT[hp][p0:p0 + 64,
                                               256 * b:256 * (b + 1)],
                                        start=(half == 0), stop=(half == 1))
                                pt = ptp.tile([128, 512], BF16,
                                              tag=f"pt{c2}", name="pt")
                                nc.scalar.activation(pt[:], sc[:], AF.Exp)
                                pts.append(pt)
                            # band: keep iff base + cm*key + iota >= 0 with
                            # iota = [[chunk-jump, 2], [st, 256]] per tile.
                            nc.gpsimd.affine_select(
                                pts[0][:], pts[0][:],
                                pattern=[[128, 2], [-1, 256]],
                                compare_op=ALU.is_ge, fill=0.0, base=0,
                                channel_multiplier=1)
                            nc.gpsimd.affine_select(
                                pts[2][:], pts[2][:],
                                pattern=[[-128, 2], [1, 256]],
                                compare_op=ALU.is_ge, fill=0.0, base=0,
                                channel_multiplier=-1)
                            pv = pvp.tile([65, 256], F32, tag="pv",
                                          name="pv")
                            for c in range(6):
                                nc.tensor.matmul(
                                    pv[:],
                                    vT[2 * b + c][:, 65 * h:65 * h + 65],
                                    pts[c // 2][:, 256 * (c % 2):
                                                256 * (c % 2 + 1)],
                                    start=(c == 0), stop=(c == 5))
                            rh = ptp.tile([1, 256], F32R, tag="rh",
                                          name="rh")
                            with nc.allow_low_precision(
                                    reason="fp32r rounding is fine here"):
                                nc.vector.reciprocal(rh[:], pv[64:65, :])
                            rb = pvp.tile([64, 256], F32, tag="rb",
                                          name="rb")
                            nc.tensor.matmul(rb[:], ones_row[0:1, 0:64],
                                             rh[:])
                            nm = ptp.tile([64, 256], F32, tag="nm",
                                          name="nm")
                            nc.vector.tensor_copy(nm[:], pv[0:64, :])
                            with nc.allow_low_precision(
                                    reason="fp8 attn operand"):
                                nc.vector.tensor_mul(
                                    attn8[h // 4][
                                        64 * (h % 2):64 * (h % 2) + 64,
                                        (h % 4) // 2:(h % 4) // 2 + 1,
                                        256 * b:256 * (b + 1)],
                                    nm[:], rb[:])

                # ---- output projection + residual (reuse qT slots) ----
                x_sb = [qat.tile([128, T], BF16, tag=f"qT{m}", name=f"xr{m}")
                        for m in range(8)]
                with tc.tile_pool(name="ops", bufs=4, space="PSUM") as ops, \
                     tc.tile_pool(name="otp", bufs=4) as otp:
                    for t2 in range(2):
                        for m in range(8):
                            ps = ops.tile([128, 512], F32, tag="po",
                                          name="po")
                            for g in range(4):
                                nc.tensor.matmul(
                                    ps[:],
                                    wo_sb[g][:, :, 128 * m:128 * (m + 1)],
                                    attn8[g][:, :, 512 * t2:512 * (t2 + 1)],
                                    start=(g == 0), stop=(g == 3),
                                    perf_mode=DR)
                            ot = otp.tile([128, 512], F32, tag="ot",
                                          name="ot")
                            nc.scalar.activation(ot[:], ps[:], AF.Identity,
                                                 scale=1.0 / 16,
                                                 bias=bt["bo"][:, m:m + 1])
                            nc.vector.tensor_add(
                                x_sb[m][:, 512 * t2:512 * (t2 + 1)], ot[:],
                                xC[m][:, 512 * t2:512 * (t2 + 1)])

        # ---- LN1 + FFN + LN2, interleaved so the in-order PE queue never
        # waits on LN stats: broadcast matmuls are placed a few FFN groups
        # deep, by which point their row inputs are long ready. ----
        x1 = [qat.tile([128, T], BF16, tag=f"aT{m}", name=f"x1{m}")
              for m in range(8)]
        with tc.tile_pool(name="lnp", bufs=2) as lnp, \
             tc.tile_pool(name="h1p", bufs=64) as h1p, \
             tc.tile_pool(name="wfp", bufs=5) as wfp, \
             tc.tile_pool(name="x2p", bufs=16) as x2p, \
             tc.tile_pool(name="fps", bufs=1, space="PSUM") as fps:
            h1 = {}
            x2 = {}

            def ffn1(t2, m1s):
                for m1 in m1s:
                    w1m = wfp.tile([128, 1024], BF16, tag="wfm", name="w1m")
                    nc.sync.dma_start(
                        w1m[:], w1_d.ap()[:, 1024 * m1:1024 * (m1 + 1)])
                    ps = fps.tile([128, 512], F32, tag="f1", bufs=2,
                                  name="f1")
                    for k in range(8):
                        nc.tensor.matmul(
                            ps[:], w1m[:, 128 * k:128 * (k + 1)],
                            x1[k][:, 512 * t2:512 * (t2 + 1)],
                            start=(k == 0), stop=(k == 7))
                    ht = h1p.tile([128, 512], BF16, tag="h1", name="h1")
                    nc.scalar.activation(ht[:], ps[:], AF.Gelu,
                                         bias=bt["b1"][:, m1:m1 + 1])
                    h1[(m1, t2)] = ht

            def ffn2(t2, m2s):
                for m2 in m2s:
                    ps = fps.tile([128, 512], F32, tag="f1", bufs=2,
                                  name="f2")
                    for kg in range(4):
                        w2m = wfp.tile([128, 1024], BF16, tag="wfm",
                                       name="w2m")
                        nc.sync.dma_start(
                            w2m[:],
                            w2_d.ap()[:, 1024 * (4 * m2 + kg):
                                      1024 * (4 * m2 + kg + 1)])
                        for k in range(8):
                            nc.tensor.matmul(
                                ps[:], w2m[:, 128 * k:128 * (k + 1)],
                                h1[(8 * kg + k, t2)][:],
                                start=(kg == 0 and k == 0),
                                stop=(kg == 3 and k == 7))
                    xt2 = x2p.tile([128, 512], F32R, tag="x2", name="x2")
                    nc.vector.scalar_tensor_tensor(
                        xt2[:], ps[:], bt["b2"][:, m2:m2 + 1],
                        x1[m2][:, 512 * t2:512 * (t2 + 1)],
                        ALU.add, ALU.add)
                    x2[(m2, t2)] = xt2

            sl = [slice(0, 512), slice(512, 1024)]
            st1 = []
            for t2 in range(2):
                st1.append(ln_stats(fps, lnp,
                                    lambda m, t2=t2: x_sb[m][:, sl[t2]],
                                    ones_colb, ones_col, eps1, sfx=f"{t2}"))
            mb0, rb0 = ln_bcast(fps, st1[0], ones_row)
            ln_apply(lnp, lambda m: x_sb[m][:, sl[0]], bt["g1"], bt["e1"],
                     lambda m: x1[m][:, sl[0]], mb0, rb0)
            ffn1(0, range(0, 12))
            mb1, rb1 = ln_bcast(fps, st1[1], ones_row)
            ln_apply(lnp, lambda m: x_sb[m][:, sl[1]], bt["g1"], bt["e1"],
                     lambda m: x1[m][:, sl[1]], mb1, rb1)
            ffn1(0, range(12, 32))
            ffn1(1, range(0, 32))
            ffn2(0, range(0, 8))
            stb0 = ln_stats(fps, lnp, lambda m: x2[(m, 0)][:], ones_col,
                            ones_col, eps1, sfx="0")
            ffn2(1, range(0, 2))
            mbb0, rbb0 = ln_bcast(fps, stb0, ones_row)
            yt0 = [x2p.tile([128, 512], F32, tag="yt", bufs=4,
                            name=f"yt{m}") for m in range(8)]
            ln_apply(lnp, lambda m: x2[(m, 0)][:], bt["g2"], bt["e2"],
                     lambda m: yt0[m][:], mbb0, rbb0)
            for m in range(8):
                nc.sync.dma_start(yT_d.ap()[128 * m:128 * (m + 1), sl[0]],
                                  yt0[m][:])
            ffn2(1, range(2, 8))
            stb1 = ln_stats(fps, lnp, lambda m: x2[(m, 1)][:], ones_col,
                            ones_col, eps1, sfx="1")
            mbb1, rbb1 = ln_bcast(fps, stb1, ones_row)
            yt1 = [x2p.tile([128, 512], F32, tag="yt", bufs=4,
                            name=f"yt{m}") for m in range(8)]
            ln_apply(lnp, lambda m: x2[(m, 1)][:], bt["g2"], bt["e2"],
                     lambda m: yt1[m][:], mbb1, rbb1)
            for m in range(8):
                nc.sync.dma_start(yT_d.ap()[128 * m:128 * (m + 1), sl[1]],
                                  yt1[m][:])

    nc.compile()
    return nc


def _host_prep(inputs):
    import ml_dtypes
    BF = ml_dtypes.bfloat16
    F8 = ml_dtypes.float8_e4m3

    def pack_dr(w):
        # [p, g*2F + i*F + n] = (16*w)[g*256 + i*128 + p, n], fp8
        wn = np.asarray(w, np.float32) * 16.0
        out = wn.reshape(4, 2, 128, wn.shape[1]).transpose(2, 0, 1, 3)
        return np.ascontiguousarray(out.reshape(128, -1)).astype(F8)

    hs = np.asarray(inputs["hidden_states"], np.float32)
    am = np.asarray(inputs["attention_mask"], np.float32)
    hm = np.asarray(inputs["layer_head_mask"], np.float32)
    sc = 1.0 / np.sqrt(HD)
    bq = np.asarray(inputs["bq"], np.float32) * sc
    wo = np.asarray(inputs["Wo"], np.float32) * np.repeat(hm, HD)[:, None]

    def tile_bias(b, ncol):
        return np.ascontiguousarray(b.reshape(ncol, 128).T)

    bv = np.asarray(inputs["bv"], np.float32)
    bvB = np.zeros((128, H * 65), np.float32)
    for h in range(H):
        bvB[:, 65 * h:65 * h + 64] = bv[64 * h:64 * h + 64][None, :]
        bvB[:, 65 * h + 64] = 1.0

    w1 = np.asarray(inputs["W1"], np.float32)
    w2 = np.asarray(inputs["W2"], np.float32)
    # w1p[p, m1*1024 + ko*128 + n] = W1[ko*128+p, m1*128+n]
    w1p = w1.reshape(8, 128, 32, 128).transpose(1, 2, 0, 3).reshape(128, -1)
    # w2p[p, (m2*4+kg)*1024 + ko*128 + n] = W2[kg*1024+ko*128+p, m2*128+n]
    w2p = w2.reshape(4, 8, 128, 8, 128).transpose(2, 3, 0, 1, 4).reshape(
        128, -1)

    biasblk = np.concatenate(
        [tile_bias(bq, 8),
         tile_bias(np.asarray(inputs["bk"], np.float32), 8),
         tile_bias(np.asarray(inputs["bo"], np.float32), 8),
         tile_bias(np.asarray(inputs["b1"], np.float32), 32),
         tile_bias(np.asarray(inputs["b2"], np.float32), 8),
         tile_bias(np.asarray(inputs["ln1_g"], np.float32), 8),
         tile_bias(np.asarray(inputs["ln1_b"], np.float32), 8),
         tile_bias(np.asarray(inputs["ln2_g"], np.float32), 8),
         tile_bias(np.asarray(inputs["ln2_b"], np.float32), 8)], axis=1)
    common = {
        "wq": pack_dr(np.asarray(inputs["Wq"], np.float32)),
        "wk": pack_dr(np.asarray(inputs["Wk"], np.float32)),
        "wv": pack_dr(np.asarray(inputs["Wv"], np.float32)),
        "wo": pack_dr(wo),
        "w1": np.ascontiguousarray(w1p).astype(BF),
        "w2": np.ascontiguousarray(w2p).astype(BF),
    }
    in_maps = []
    for core in range(NCORES):
        b, s0 = core // 4, (core % 4) * T
        lo, hi = s0 - W, s0 + T + W
        a, c = max(lo, 0), min(hi, S)
        xh = np.zeros((TH, D), np.float32)
        xh[a - lo:c - lo] = hs[b, a:c]
        km = np.full((TH,), NEG, np.float32)
        km[a - lo:c - lo] = am[b, a:c]
        m = dict(common)
        m["onesP"] = np.ones((128, 1), np.float32)
        m["onesF"] = np.ones((1, 128), np.float32)
        vmask = np.exp(np.minimum(km, 0.0).astype(np.float64)).astype(
            np.float32)
        m["cstb"] = np.ascontiguousarray(np.concatenate(
            [np.ascontiguousarray(vmask.reshape(NCH, 128).T), bvB, biasblk],
            axis=1))
        xt = xh.T  # [D, TH]
        m["xc"] = np.ascontiguousarray(xt[:, W:W + T]).astype(BF)
        m["x8"] = np.ascontiguousarray(
            xt.reshape(4, 2, 128, TH).transpose(2, 0, 1, 3).reshape(
                128, -1)).astype(F8)
        in_maps.append(m)
    return in_maps


def kernel(**inputs):
    from concourse.bass_utils import run_bass_kernel_spmd

    if "nc" not in _CACHE:
        _CACHE["nc"] = _build()
    nc = _CACHE["nc"]
    in_maps = _host_prep(inputs)
    res = run_bass_kernel_spmd(nc, in_maps, core_ids=list(range(NCORES)))
    _CACHE["exec_time_ns"] = res.exec_time_ns
    out = np.zeros((B, S, D), np.float32)
    for core in range(NCORES):
        b, s0 = core // 4, (core % 4) * T
        out[b, s0:s0 + T] = res.results[core]["yT"].T
    return out
